# revision 12
# baseline (speedup 1.0000x reference)
"""Trainium2 Bass kernel for the BANLayer problem.

Data-parallel over batch: B=64 sharded as 8 batches per NeuronCore x 8 cores.
All params replicated. Host pre-transposes q/v to feature-major layout and
folds weight-norm into the weights so the device kernel is pure matmul+softmax.

Math (per batch b):
  vp  = relu(v @ Wv^T + vb)                 [VN, HK]
  qp  = relu(q @ Wq^T + qb)                 [QN, HK]
  att_h = (vp * h_mat[h]) @ qp^T            [VN, QN]  (h = 0, 1)
  softmax over joint (VN*QN) with valid-mask; att output = probs
  A = sum_h att_h
  fusion[k] = sum_v sum_q vp[v,k] A[v,q] qp[q,k]
  logits = LN(groupsum3(fusion))

Masking trick: accumulate BIG * vmask x qmask into the attention PSUM via a
K=1 matmul; exp(x - max) then sends invalid entries (no +BIG) to exactly 0.
"""

import sys

sys.path.insert(0, "/opt/trn_rl_repo")

import numpy as np

import concourse.bass as bass
import concourse.tile as tile
from concourse import bacc, bass_isa, mybir
from concourse import bass_utils

F32 = mybir.dt.float32
F32R = mybir.dt.float32r
AX = mybir.AxisListType
ALU = mybir.AluOpType
ACTF = mybir.ActivationFunctionType

N_CORES = 8
B_LOC = 8          # batches per core
VN, QN = 256, 1024
VD, QD = 128, 1280
HK, HD, HO, KGRP = 768, 256, 2, 3
HKC = HK // 128    # 6
QDC = QD // 128    # 10
BIG = 1024.0
LN_EPS = 1e-5
SHIFT_C = 16.0

_cache = {}


def _f32(ap):
    return ap.bitcast(F32)


def _build(softmax, h_bias_vals):
    nc = bacc.Bacc(
        "TRN2",
        target_bir_lowering=False,
        debug=False,
        enable_asserts=False,
        num_devices=N_CORES,
        enable_partition_id=False,
    )

    d_qt = nc.dram_tensor("qt", [B_LOC, QD, QN], F32R, kind="ExternalInput").ap()
    d_vt = nc.dram_tensor("vt", [B_LOC, VD, VN], F32R, kind="ExternalInput").ap()
    d_vm = nc.dram_tensor("vm", [B_LOC, VN], F32R, kind="ExternalInput").ap()
    d_qm = nc.dram_tensor("qm", [B_LOC, QN], F32R, kind="ExternalInput").ap()
    d_wq = nc.dram_tensor("wq", [128, QDC, HK], F32R, kind="ExternalInput").ap()
    d_wv = nc.dram_tensor("wv", [VD, HK], F32R, kind="ExternalInput").ap()
    d_vbc = nc.dram_tensor("vbc", [128, HKC], F32, kind="ExternalInput").ap()
    d_qbc = nc.dram_tensor("qbc", [128, HKC], F32, kind="ExternalInput").ap()
    d_vbr = nc.dram_tensor("vbr", [128, HK], F32, kind="ExternalInput").ap()
    d_hm = nc.dram_tensor("hm", [128, HKC, HO], F32, kind="ExternalInput").ap()
    d_gam = nc.dram_tensor("gam", [B_LOC, HD], F32, kind="ExternalInput").ap()
    d_bet = nc.dram_tensor("bet", [B_LOC, HD], F32, kind="ExternalInput").ap()

    d_att = nc.dram_tensor("att", [B_LOC, HO, VN, QN], F32, kind="ExternalOutput").ap()
    d_log = nc.dram_tensor("logits", [B_LOC, HD], F32, kind="ExternalOutput").ap()

    with tile.TileContext(nc) as tc:
        with (
            tc.tile_pool(name="const", bufs=1) as cpool,
            tc.tile_pool(name="qth", bufs=2) as qth_pool,
            tc.tile_pool(name="qpt", bufs=2) as qpt_pool,
            tc.tile_pool(name="vsmall", bufs=2) as vpool,
            tc.tile_pool(name="attm", bufs=2) as attm_pool,
            tc.tile_pool(name="apool", bufs=1) as a_pool,
            tc.tile_pool(name="small", bufs=2) as spool,
            tc.tile_pool(name="gscr", bufs=2) as gpool,
            tc.tile_pool(name="fin", bufs=1) as fin_pool,
            tc.tile_pool(name="pp", bufs=2, space="PSUM") as pp,
            tc.tile_pool(name="dscr", bufs=1, space="DRAM") as dpool,
        ):
            # ---- constants (loaded once) ----
            wq_sb = cpool.tile([128, QDC, HK], F32R, name="wq_sb")
            nc.sync.dma_start(out=wq_sb, in_=d_wq)
            wv_sb = cpool.tile([VD, HK], F32R, name="wv_sb")
            nc.sync.dma_start(out=wv_sb, in_=d_wv)
            vb_col = cpool.tile([128, HKC], F32, name="vb_col")
            nc.sync.dma_start(out=vb_col, in_=d_vbc)
            qb_col = cpool.tile([128, HKC], F32, name="qb_col")
            nc.sync.dma_start(out=qb_col, in_=d_qbc)
            vb_rep = cpool.tile([128, HK], F32, name="vb_rep")
            nc.sync.dma_start(out=vb_rep, in_=d_vbr)
            hm_sb = cpool.tile([128, HKC, HO], F32, name="hm_sb")
            nc.sync.dma_start(out=hm_sb, in_=d_hm)
            gam_sb = cpool.tile([B_LOC, HD], F32, name="gam_sb")
            nc.sync.dma_start(out=gam_sb, in_=d_gam)
            bet_sb = cpool.tile([B_LOC, HD], F32, name="bet_sb")
            nc.sync.dma_start(out=bet_sb, in_=d_bet)
            bias_c = cpool.tile([128, 1], F32, name="bias_c")
            nc.vector.memset(bias_c, -(BIG + SHIFT_C))
            eps_t = cpool.tile([B_LOC, 1], F32, name="eps_t")
            nc.vector.memset(eps_t, LN_EPS)
            # fusion accumulators for all batches: [:, b, c, qn]
            fus_all = cpool.tile([128, B_LOC, HKC, 2], F32, name="fus_all")

            for b in range(B_LOC):
                # ---- per-batch small loads ----
                vt_sb = vpool.tile([VD, VN], F32R, name="vt_sb", tag="vt")
                nc.sync.dma_start(out=vt_sb, in_=d_vt[b])
                vm_sb = vpool.tile([1, VN], F32R, name="vm_sb", tag="vm", bufs=1)
                nc.sync.dma_start(out=vm_sb, in_=d_vm[b].unsqueeze(0))
                qm_sb = vpool.tile([1, QN], F32R, name="qm_sb", tag="qm", bufs=1)
                nc.sync.dma_start(out=qm_sb, in_=d_qm[b].unsqueeze(0))

                # ---- vpT [hk, v] = relu(Wv^T-chunks . vT + vb) ----
                vpt = vpool.tile([128, HKC, VN], F32, name="vpt", tag="vpt")
                for c in range(HKC):
                    ps = pp.tile([128, 512], F32, name="ps_vpt", tag="misc", bufs=1)
                    nc.tensor.matmul(
                        ps[:, :VN],
                        (wv_sb[:, c * 128 : (c + 1) * 128]),
                        (vt_sb),
                        start=True,
                        stop=True,
                    )
                    nc.scalar.activation(
                        out=vpt[:, c, :],
                        in_=ps[:, :VN],
                        func=ACTF.Relu,
                        bias=vb_col[:, c : c + 1],
                        scale=1.0,
                    )

                # ---- vp [v, hk] = relu(vT-chunks^T . Wv^T + vb) ----
                vp = vpool.tile([128, 2, HK], F32R, name="vp", tag="vp")
                for vc in range(2):
                    for n0, n1 in ((0, 512), (512, HK)):
                        ps = pp.tile([128, 512], F32, name="ps_vp", tag="misc", bufs=1)
                        nc.tensor.matmul(
                            ps[:, : n1 - n0],
                            (vt_sb[:, vc * 128 : (vc + 1) * 128]),
                            (wv_sb[:, n0:n1]),
                            start=True,
                            stop=True,
                        )
                        nc.vector.tensor_add(
                            vp[:, vc, n0:n1], ps[:, : n1 - n0], vb_rep[:, n0:n1]
                        )
                    nc.vector.tensor_scalar_max(
                        vp[:, vc, :], _f32(vp[:, vc, :]), 0.0
                    )

                # ---- qpT [hk, q] = relu(Wq^T-chunks . qT + qb) ----
                qpt = qpt_pool.tile([128, HKC, QN], F32R, name="qpt", tag="qpt")
                for qh in range(2):
                    qth = qth_pool.tile([128, QDC, 512], F32R, name="qth", tag="qth")
                    nc.sync.dma_start(
                        out=qth,
                        in_=d_qt[b][:, qh * 512 : (qh + 1) * 512].rearrange(
                            "(dc p) n -> p dc n", p=128
                        ),
                    )
                    for c in range(HKC):
                        ps = pp.tile([128, 512], F32, name="ps_proj", tag="proj")
                        for dc in range(QDC):
                            nc.tensor.matmul(
                                ps,
                                (wq_sb[:, dc, c * 128 : (c + 1) * 128]),
                                (qth[:, dc, :]),
                                start=(dc == 0),
                                stop=(dc == QDC - 1),
                            )
                        nc.scalar.activation(
                            out=qpt[:, c, qh * 512 : (qh + 1) * 512],
                            in_=ps,
                            func=ACTF.Relu,
                            bias=qb_col[:, c : c + 1],
                            scale=1.0,
                        )

                # ---- attention per head ----
                attms = []
                if softmax:
                    sumc4 = spool.tile([128, HO, 4], F32, name="sumc4", tag="sumc4")
                    srd = spool.tile([128, HO], F32, name="srd", tag="srd")
                    rcol = spool.tile([128, HO], F32, name="rcol", tag="rcol")
                else:
                    # multiplicative valid mask [v, q] via K=1 outer product
                    valid_sb = a_pool.tile([128, 2, QN], F32, name="valid_sb", tag="valid")
                    for vc in range(2):
                        for qn in range(2):
                            ps = pp.tile([128, 512], F32, name="ps_msk", tag="misc", bufs=1)
                            nc.tensor.matmul(
                                ps,
                                (vm_sb[:, vc * 128 : (vc + 1) * 128]),
                                (qm_sb[:, qn * 512 : (qn + 1) * 512]),
                                start=True,
                                stop=True,
                            )
                            nc.vector.tensor_copy(
                                valid_sb[:, vc, qn * 512 : (qn + 1) * 512], ps
                            )

                for h in range(HO):
                    vph = vpool.tile([128, HKC, VN], F32R, name="vph", tag="vph")
                    for c in range(HKC):
                        nc.vector.tensor_scalar_mul(
                            vph[:, c, :], vpt[:, c, :], hm_sb[:, c, h : h + 1]
                        )
                    attm = attm_pool.tile([128, 2, QN], F32, name="attm", tag="attm")
                    attms.append(attm)
                    for vc in range(2):
                        for qn in range(2):
                            ps = pp.tile([128, 512], F32, name="ps_att", tag="att", bufs=3)
                            for c in range(HKC):
                                nc.tensor.matmul(
                                    ps,
                                    (vph[:, c, vc * 128 : (vc + 1) * 128]),
                                    (qpt[:, c, qn * 512 : (qn + 1) * 512]),
                                    start=(c == 0),
                                    stop=(not softmax) and (c == HKC - 1),
                                )
                            if softmax:
                                # += BIG * vmask x qmask
                                nc.tensor.matmul(
                                    ps,
                                    (vm_sb[:, vc * 128 : (vc + 1) * 128]),
                                    (qm_sb[:, qn * 512 : (qn + 1) * 512]),
                                    start=False,
                                    stop=True,
                                )
                                nc.scalar.activation(
                                    out=attm[:, vc, qn * 512 : (qn + 1) * 512],
                                    in_=ps,
                                    func=ACTF.Exp,
                                    bias=bias_c,
                                    scale=1.0,
                                    accum_out=sumc4[:, h, 2 * vc + qn : 2 * vc + qn + 1],
                                )
                            else:
                                # (att + h_bias) * valid
                                nc.vector.scalar_tensor_tensor(
                                    out=attm[:, vc, qn * 512 : (qn + 1) * 512],
                                    in0=ps,
                                    scalar=float(h_bias_vals[h]),
                                    in1=valid_sb[:, vc, qn * 512 : (qn + 1) * 512],
                                    op0=ALU.add,
                                    op1=ALU.mult,
                                )
                    if softmax:
                        nc.vector.tensor_reduce(
                            out=srd[:, h : h + 1],
                            in_=sumc4[:, h, :],
                            axis=AX.X,
                            op=ALU.add,
                        )
                        nc.gpsimd.partition_all_reduce(
                            srd[:, h : h + 1],
                            srd[:, h : h + 1],
                            channels=128,
                            reduce_op=bass_isa.ReduceOp.add,
                        )
                        nc.vector.reciprocal(rcol[:, h : h + 1], srd[:, h : h + 1])
                        nc.scalar.mul(attms[h], attms[h], rcol[:, h : h + 1])

                for h in range(HO):
                    nc.sync.dma_start(
                        out=d_att[b, h].rearrange("(vc p) q -> p vc q", p=128),
                        in_=attms[h],
                    )

                # ---- A = att_0 + att_1 ; E^T and fusion reduction ----
                a_sb = a_pool.tile([128, 2, QN], F32R, name="a_sb", tag="a")
                nc.vector.tensor_add(a_sb, attms[0], attms[1])
                for c in range(HKC):
                    for qn in range(2):
                        ps = pp.tile([128, 512], F32, name="ps_et", tag="et")
                        for vc in range(2):
                            nc.tensor.matmul(
                                ps,
                                (vp[:, vc, c * 128 : (c + 1) * 128]),
                                (a_sb[:, vc, qn * 512 : (qn + 1) * 512]),
                                start=(vc == 0),
                                stop=(vc == 1),
                            )
                        g_scr = gpool.tile([128, 512], F32, name="g_scr", tag="g")
                        nc.vector.tensor_mul(
                            g_scr, ps, _f32(qpt[:, c, qn * 512 : (qn + 1) * 512])
                        )
                        nc.scalar.activation(
                            out=g_scr,
                            in_=g_scr,
                            func=ACTF.Copy,
                            accum_out=fus_all[:, b, c, qn : qn + 1],
                        )

            # ---- logits: flatten fusion, group-sum, layernorm ----
            fus_red = fin_pool.tile([128, B_LOC, HKC], F32, name="fus_red")
            nc.vector.tensor_reduce(
                out=fus_red, in_=fus_all, axis=AX.X, op=ALU.add
            )
            scr = dpool.tile([B_LOC * HK], F32, name="scr")
            nc.sync.dma_start(
                out=scr.rearrange("(b c p) -> p b c", p=128, c=HKC), in_=fus_red
            )
            fus_flat = fin_pool.tile([B_LOC, HK], F32, name="fus_flat")
            nc.sync.dma_start(out=fus_flat, in_=scr.rearrange("(b k) -> b k", k=HK))
            lraw = fin_pool.tile([B_LOC, HD], F32, name="lraw")
            nc.vector.tensor_reduce(
                out=lraw,
                in_=fus_flat.rearrange("b (j k) -> b j k", k=KGRP),
                axis=AX.X,
                op=ALU.add,
            )
            stats = fin_pool.tile([B_LOC, 6], F32, name="stats")
            nc.vector.bn_stats(out=stats, in_=lraw)
            mv = fin_pool.tile([B_LOC, 2], F32, name="mv")
            nc.vector.bn_aggr(out=mv, in_=stats)
            sd = fin_pool.tile([B_LOC, 1], F32, name="sd")
            nc.scalar.activation(
                out=sd, in_=mv[:, 1:2], func=ACTF.Sqrt, bias=eps_t, scale=1.0
            )
            rs = fin_pool.tile([B_LOC, 1], F32, name="rs")
            nc.vector.reciprocal(rs, sd)
            lout = fin_pool.tile([B_LOC, HD], F32, name="lout")
            nc.vector.tensor_scalar(
                out=lout,
                in0=lraw,
                scalar1=mv[:, 0:1],
                scalar2=rs,
                op0=ALU.subtract,
                op1=ALU.mult,
            )
            nc.vector.tensor_mul(lout, lout, gam_sb)
            nc.vector.tensor_add(lout, lout, bet_sb)
            nc.sync.dma_start(out=d_log, in_=lout)

    nc.compile()
    return nc


def _get_program(softmax, h_bias_vals):
    key = (int(softmax), tuple(np.asarray(h_bias_vals, dtype=np.float64).tolist()))
    if key not in _cache:
        _cache[key] = _build(int(softmax), np.asarray(h_bias_vals, dtype=np.float32))
    return _cache[key]


def kernel(v, q, v_mask, q_mask, softmax, v_w, v_g, v_b, q_w, q_g, q_b,
           h_mat, h_bias, ln_gamma, ln_beta):
    v = np.asarray(v, np.float32)
    q = np.asarray(q, np.float32)
    sm = int(np.asarray(softmax))
    B = v.shape[0]
    assert B == N_CORES * B_LOC

    # ---- host-side param folding & layout prep ----
    wnv = (np.float32(v_g) * np.asarray(v_w, np.float32)
           / np.sqrt(np.sum(np.asarray(v_w, np.float64) ** 2, dtype=np.float64)).astype(np.float32))
    wnq = (np.float32(q_g) * np.asarray(q_w, np.float32)
           / np.sqrt(np.sum(np.asarray(q_w, np.float64) ** 2, dtype=np.float64)).astype(np.float32))
    wv_t = np.ascontiguousarray(wnv.T)                       # [128, 768]
    wq_t = np.ascontiguousarray(wnq.T)                       # [1280, 768]
    wq_sb = np.ascontiguousarray(
        wq_t.reshape(QDC, 128, HK).transpose(1, 0, 2))       # [128, 10, 768]
    vb_col = np.ascontiguousarray(
        np.asarray(v_b, np.float32).reshape(HKC, 128).T)     # [128, 6]
    qb_col = np.ascontiguousarray(
        np.asarray(q_b, np.float32).reshape(HKC, 128).T)     # [128, 6]
    vb_rep = np.ascontiguousarray(
        np.tile(np.asarray(v_b, np.float32)[None, :], (128, 1)))  # [128, 768]
    hm_sb = np.ascontiguousarray(
        np.asarray(h_mat, np.float32).T.reshape(HKC, 128, HO).transpose(1, 0, 2))
    gam8 = np.ascontiguousarray(
        np.tile(np.asarray(ln_gamma, np.float32)[None, :], (B_LOC, 1)))
    bet8 = np.ascontiguousarray(
        np.tile(np.asarray(ln_beta, np.float32)[None, :], (B_LOC, 1)))

    qT = np.ascontiguousarray(q.transpose(0, 2, 1))          # [64, 1280, 1024]
    vT = np.ascontiguousarray(v.transpose(0, 2, 1))          # [64, 128, 256]
    vm_s = np.asarray(v_mask, np.float32) * (BIG if sm else 1.0)
    qm_f = np.asarray(q_mask, np.float32)

    nc = _get_program(sm, np.asarray(h_bias, np.float32))

    in_maps = []
    for c in range(N_CORES):
        s = slice(c * B_LOC, (c + 1) * B_LOC)
        in_maps.append({
            "qt": qT[s], "vt": vT[s], "vm": vm_s[s], "qm": qm_f[s],
            "wq": wq_sb, "wv": wv_t, "vbc": vb_col, "qbc": qb_col,
            "vbr": vb_rep, "hm": hm_sb, "gam": gam8, "bet": bet8,
        })

    import os
    trace = bool(int(os.environ.get("BAN_TRACE", "0")))
    res = bass_utils.run_bass_kernel_spmd(
        nc, in_maps, core_ids=list(range(N_CORES)), trace=trace,
        tmpdir=os.environ.get("BAN_TRACE_DIR") or None,
    )
    globals()["last_res"] = res
    logits = np.concatenate([r["logits"] for r in res.results], axis=0)
    att = np.concatenate([r["att"] for r in res.results], axis=0)
    return logits, att
